# revision 15
# baseline (speedup 1.0000x reference)
"""LocalPushPlus loss kernel for 8 Trainium2 NeuronCores.

Strategy (data-parallel over rows, per sharding hint):
  - Host: permute rows so each core owns 1024 rows consisting of WHOLE label
    classes (classes never straddle cores, contiguous within a core).  The
    final loss is permutation-invariant, so this is a pure resharding.
  - Device (SPMD, identical program, per-core data):
      1. Normalize own 1024 feature rows (norms from row-major shard), compute
         the positive similarity a_i = f_i . c_{l_i} / (|f_i||c_{l_i}|) via an
         indirect-DMA gather of center rows.
      2. Scale own transposed shard fT[:, own] by 1/|f_j| (bf16) and AllGather
         the normalized transposed features so every core holds fT_n [512, 8192].
      3. S block = own_cols^T @ fT_n on the PE (bf16, fp32 PSUM).  ScalarE
         applies relu(S - a_i) with per-partition bias and accumulates the
         row sums; VectorE counts positives (4x fp16 mode).
      4. Same-class pairs are contiguous (sorting) so they fall in a +/-64
         column window around the diagonal; small local matmuls over the
         zero-padded own shard recompute those sims bit-exactly and DVE
         subtracts their (masked) sum/count.
      5. Per-row sample losses, per-core partial sum + positive count;
         host combines 8 partial pairs into the final scalar.
"""

import numpy as np
from contextlib import ExitStack

import concourse.bass as bass
import concourse.bacc as bacc
import concourse.tile as tile
from concourse import mybir
from concourse.bass_utils import run_bass_kernel_spmd

F32 = mybir.dt.float32
F16 = mybir.dt.float16
BF16 = mybir.dt.bfloat16
I32 = mybir.dt.int32
ALU = mybir.AluOpType
ACTF = mybir.ActivationFunctionType
AX = mybir.AxisListType

N, D, C = 8192, 512, 1000
M = 8            # cores
NL = N // M      # 1024 rows per core
P = 128
NT = NL // P     # 8 row tiles per core
KT = D // P      # 4 contraction tiles
WIN = 256        # same-class correction window
PAD = 64         # zero pad on each side of the own-shard columns
GRP = 2048       # PSUM group width (4 banks)
NG = N // GRP    # 4 groups per row tile

_prog_cache = {}

# test harness hooks: extra kwargs for run_bass_kernel_spmd (e.g. trace=True)
# and the last BassKernelResults for timing inspection.
RUN_KWARGS = {}
LAST_RESULT = None


def _bcast_ap(ap, parts):
    """Partition-broadcast (stride 0) view of a 1-D slice."""
    inner = ap.ap[-1]
    return bass.AP(tensor=ap.tensor, offset=ap.offset, ap=[[0, parts], inner])


def _build_program():
    nc = bacc.Bacc("TRN2", num_devices=M)

    fsh_h = nc.declare_dram_parameter("fsh", [NL, D], F32, isOutput=False)
    ftsh_h = nc.declare_dram_parameter("fTsh", [D, NL], F32, isOutput=False)
    cen_h = nc.declare_dram_parameter("cen", [C, D], F32, isOutput=False)
    labf_h = nc.declare_dram_parameter("labf", [NL], F32, isOutput=False)
    labi_h = nc.declare_dram_parameter("labi", [NL], I32, isOutput=False)
    winl_h = nc.declare_dram_parameter("winl", [NT, WIN], F32, isOutput=False)
    part_h = nc.declare_dram_parameter("partial", [1, 2], F32, isOutput=True)

    r_dram = nc.dram_tensor("r_dram", [NL], F32)
    ftn_loc = nc.dram_tensor("ftn_loc", [D, NL], BF16)
    ftn_all = nc.dram_tensor("ftn_all", [M, D, NL], BF16, addr_space="Shared")

    with tile.TileContext(nc, num_cores=M) as tc, ExitStack() as ctx:
        singles = ctx.enter_context(tc.tile_pool(name="singles", bufs=1))
        pa = ctx.enter_context(tc.tile_pool(name="phasea", bufs=2))
        ps_small = ctx.enter_context(tc.tile_pool(name="small", bufs=16))

        bias_all = singles.tile([P, NT], F32)   # -a_i
        rwork = singles.tile([P, NT], F32)      # 1/|f_i|
        sl_all = singles.tile([P, NT], F32)
        pos_all = singles.tile([P, NT], F32)
        labL = singles.tile([P, NT], F32)
        labI = singles.tile([P, NT], I32)
        corr_s = singles.tile([P, NT], F32)
        corr_c = singles.tile([P, NT], F32)
        c1_all = singles.tile([P, NT], F32)
        rb = singles.tile([P, NL], BF16)        # 1/|f_j| broadcast (own cols)

        nc.sync.dma_start(out=labL, in_=labf_h[:].rearrange("(t p) -> p t", p=P))
        nc.sync.dma_start(out=labI, in_=labi_h[:].rearrange("(t p) -> p t", p=P))

        # ---- Phase A: norms, positive similarities, per-row bias ----
        for t in range(NT):
            fsh_t = pa.tile([P, D], F32, tag="fsh")
            nc.sync.dma_start(out=fsh_t, in_=fsh_h[t * P:(t + 1) * P, :])
            cg_t = pa.tile([P, D], F32, tag="cg")
            nc.gpsimd.indirect_dma_start(
                out=cg_t[:],
                out_offset=None,
                in_=cen_h[:],
                in_offset=bass.IndirectOffsetOnAxis(ap=labI[:, t:t + 1], axis=0),
            )
            sqf = pa.tile([P, D], F32, tag="sqf")
            n2f = ps_small.tile([P, 1], F32, tag="n2f")
            nc.scalar.activation(out=sqf, in_=fsh_t, func=ACTF.Square, accum_out=n2f)
            sqc = pa.tile([P, D], F32, tag="sqc")
            n2c = ps_small.tile([P, 1], F32, tag="n2c")
            nc.scalar.activation(out=sqc, in_=cg_t, func=ACTF.Square, accum_out=n2c)
            dot = pa.tile([P, D], F32, tag="dot")
            araw = ps_small.tile([P, 1], F32, tag="araw")
            nc.vector.scalar_tensor_tensor(
                out=dot, in0=fsh_t, scalar=1.0, in1=cg_t,
                op0=ALU.mult, op1=ALU.mult, accum_out=araw,
            )
            i2f = ps_small.tile([P, 1], F32, tag="i2f")
            nc.vector.reciprocal(out=i2f, in_=n2f)
            i2c = ps_small.tile([P, 1], F32, tag="i2c")
            nc.vector.reciprocal(out=i2c, in_=n2c)
            nc.scalar.activation(out=rwork[:, t:t + 1], in_=i2f, func=ACTF.Sqrt)
            pr = ps_small.tile([P, 1], F32, tag="pr")
            nc.vector.tensor_tensor(out=pr, in0=i2f, in1=i2c, op=ALU.mult)
            sr = ps_small.tile([P, 1], F32, tag="sr")
            nc.scalar.activation(out=sr, in_=pr, func=ACTF.Sqrt)
            # bias = -a_i = araw * sr * (-1)
            nc.vector.tensor_scalar(
                out=bias_all[:, t:t + 1], in0=araw,
                scalar1=sr, scalar2=-1.0, op0=ALU.mult, op1=ALU.mult,
            )

        # 1/|f| of own rows -> DRAM -> broadcast tile (bf16)
        nc.sync.dma_start(
            out=r_dram[:].rearrange("(t p) -> p t", p=P), in_=rwork)
        nc.gpsimd.dma_start(out=rb, in_=_bcast_ap(r_dram[:], P))

        # ---- normalize own transposed shard (zero-padded) + AllGather ----
        ftnsh = []
        for kt in range(KT):
            tk = singles.tile([P, PAD + NL + PAD], BF16, tag=f"ftnsh{kt}")
            nc.gpsimd.memset(tk[:, 0:PAD], 0.0)
            nc.gpsimd.memset(tk[:, PAD + NL:], 0.0)
            raw = pa.tile([P, NL], BF16, tag="ftraw")
            nc.gpsimd.dma_start(out=raw, in_=ftsh_h[kt * P:(kt + 1) * P, :])
            nc.vector.tensor_tensor(
                out=tk[:, PAD:PAD + NL], in0=raw, in1=rb, op=ALU.mult)
            nc.sync.dma_start(
                out=ftn_loc[kt * P:(kt + 1) * P, :], in_=tk[:, PAD:PAD + NL])
            ftnsh.append(tk)

        nc.gpsimd.collective_compute(
            "AllGather",
            ALU.bypass,
            replica_groups=[list(range(M))],
            ins=[ftn_loc[:]],
            outs=[ftn_all[:]],
        )

        ftn = []
        for kt in range(KT):
            g = singles.tile([P, N], BF16, tag=f"ftn{kt}")
            nc.sync.dma_start(
                out=g[:].rearrange("p (r j) -> p r j", r=M),
                in_=ftn_all[:, kt * P:(kt + 1) * P, :].rearrange("r p j -> p r j"),
            )
            ftn.append(g)

        # ---- Phase B0: same-class window corrections (local matmuls) ----
        with tc.tile_pool(name="psc", bufs=2, space="PSUM") as psc, \
             tc.tile_pool(name="winp", bufs=2) as winp:
            for t in range(NT):
                pc = psc.tile([P, WIN], F32, tag="pc")
                for kt in range(KT):
                    nc.tensor.matmul(
                        pc,
                        lhsT=ftnsh[kt][:, PAD + t * P:PAD + (t + 1) * P],
                        rhs=ftnsh[kt][:, t * P:t * P + WIN],
                        start=(kt == 0),
                        stop=(kt == KT - 1),
                    )
                rw = winp.tile([P, WIN], F16, tag="rw")
                nc.scalar.activation(
                    out=rw, in_=pc, func=ACTF.Relu, bias=bias_all[:, t:t + 1])
                wl = winp.tile([P, WIN], F16, tag="wl")
                nc.gpsimd.dma_start(out=wl, in_=_bcast_ap(winl_h[t, :], P))
                eqr = winp.tile([P, WIN], F16, tag="eqr")
                nc.vector.scalar_tensor_tensor(
                    out=eqr, in0=wl, scalar=labL[:, t:t + 1], in1=rw,
                    op0=ALU.is_equal, op1=ALU.mult,
                    accum_out=corr_s[:, t:t + 1],
                )
                nc.vector.tensor_scalar(
                    out=eqr, in0=eqr, scalar1=0.0, scalar2=None,
                    op0=ALU.is_gt, op1=ALU.add, accum_out=corr_c[:, t:t + 1],
                )

        # ---- Phase B: main S block, relu row-sums and counts ----
        with tc.tile_pool(name="psm", bufs=2, space="PSUM") as psm, \
             tc.tile_pool(name="relub", bufs=2) as relub, \
             tc.tile_pool(name="scr", bufs=4) as scr:
            for t in range(NT):
                rbuf = relub.tile([P, N], F16, tag="rbuf")
                t1g = scr.tile([P, NG], F32, tag="t1g")
                for g in range(NG):
                    ps = psm.tile([P, GRP], F32, tag="ps")
                    for jc in range(GRP // 512):
                        j0 = g * GRP + jc * 512
                        for kt in range(KT):
                            nc.tensor.matmul(
                                ps[:, jc * 512:(jc + 1) * 512],
                                lhsT=ftnsh[kt][:, PAD + t * P:PAD + (t + 1) * P],
                                rhs=ftn[kt][:, j0:j0 + 512],
                                start=(kt == 0),
                                stop=(kt == KT - 1),
                            )
                    nc.scalar.activation(
                        out=rbuf[:, g * GRP:(g + 1) * GRP], in_=ps,
                        func=ACTF.Relu, bias=bias_all[:, t:t + 1],
                        accum_out=t1g[:, g:g + 1],
                    )
                nc.vector.tensor_scalar(
                    out=rbuf, in0=rbuf, scalar1=0.0, scalar2=None,
                    op0=ALU.is_gt, op1=ALU.add, accum_out=c1_all[:, t:t + 1],
                )
                t1 = scr.tile([P, 1], F32, tag="t1")
                nc.vector.tensor_reduce(out=t1, in_=t1g, axis=AX.X, op=ALU.add)
                rs = scr.tile([P, 1], F32, tag="rs")
                nc.vector.tensor_tensor(
                    out=rs, in0=t1, in1=corr_s[:, t:t + 1], op=ALU.subtract)
                cnt = scr.tile([P, 1], F32, tag="cnt")
                nc.vector.tensor_tensor(
                    out=cnt, in0=c1_all[:, t:t + 1], in1=corr_c[:, t:t + 1],
                    op=ALU.subtract)
                nc.vector.tensor_scalar_max(out=cnt, in0=cnt, scalar1=1.0)
                rcp = scr.tile([P, 1], F32, tag="rcp")
                nc.vector.reciprocal(out=rcp, in_=cnt)
                nc.vector.tensor_tensor(
                    out=sl_all[:, t:t + 1], in0=rs, in1=rcp, op=ALU.mult)
                nc.vector.tensor_scalar(
                    out=pos_all[:, t:t + 1], in0=sl_all[:, t:t + 1],
                    scalar1=0.0, scalar2=None, op0=ALU.is_gt)

        # ---- Phase C: per-core partials (partition-reduce via ones matmul) ----
        with tc.tile_pool(name="psf", bufs=1, space="PSUM") as psf, \
             tc.tile_pool(name="fin", bufs=1) as fin:
            pc2 = fin.tile([P, 2], F32, tag="pc2")
            nc.vector.tensor_reduce(
                out=pc2[:, 0:1], in_=sl_all, axis=AX.X, op=ALU.add)
            nc.vector.tensor_reduce(
                out=pc2[:, 1:2], in_=pos_all, axis=AX.X, op=ALU.add)
            ones = fin.tile([P, 1], F32, tag="ones")
            nc.vector.memset(ones, 1.0)
            psum_out = psf.tile([1, 2], F32, tag="psum_out")
            nc.tensor.matmul(psum_out, lhsT=ones, rhs=pc2, start=True, stop=True)
            outp = fin.tile([1, 2], F32, tag="outp")
            nc.vector.tensor_copy(out=outp, in_=psum_out)
            nc.sync.dma_start(out=part_h[:], in_=outp)

    nc.finalize()
    return nc


def _get_program():
    if "nc" not in _prog_cache:
        _prog_cache["nc"] = _build_program()
    return _prog_cache["nc"]


def _pack_classes(lab):
    """Assign whole classes to cores so each core gets exactly NL rows.

    Returns a permutation of row indices (classes contiguous per core)."""
    counts = np.bincount(lab, minlength=C)
    assert counts.max() <= PAD + 1, f"class too large: {counts.max()}"
    order = np.argsort(counts)[::-1]
    caps = [NL] * M
    assign = [[] for _ in range(M)]
    for c in order:
        s = int(counts[c])
        if s == 0:
            continue
        k = max(range(M), key=lambda i: caps[i])
        if caps[k] < s:
            k = max(
                (i for i in range(M) if caps[i] >= s),
                key=lambda i: caps[i],
                default=None,
            )
            assert k is not None, "class packing failed"
        assign[k].append(c)
        caps[k] -= s
    assert all(c == 0 for c in caps), f"class packing not exact: {caps}"

    by_class = {}
    for idx, l in enumerate(lab):
        by_class.setdefault(int(l), []).append(idx)
    perm = []
    for k in range(M):
        for c in sorted(assign[k]):
            perm.extend(by_class[c])
    return np.asarray(perm, dtype=np.int64)


def kernel(features, centers, labels):
    f = np.ascontiguousarray(np.asarray(features, dtype=np.float32))
    cen = np.ascontiguousarray(np.asarray(centers, dtype=np.float32))
    lab = np.asarray(labels).astype(np.int64)

    perm = _pack_classes(lab)
    fs = f[perm]
    ls = lab[perm]
    fT = np.ascontiguousarray(fs.T)

    in_maps = []
    for r in range(M):
        sl = slice(r * NL, (r + 1) * NL)
        lloc = ls[sl]
        labpad = np.full(NL + 2 * PAD, -1.0, np.float32)
        labpad[PAD:PAD + NL] = lloc
        winl = np.stack([labpad[P * t:P * t + WIN] for t in range(NT)])
        in_maps.append({
            "fsh": np.ascontiguousarray(fs[sl]),
            "fTsh": np.ascontiguousarray(fT[:, sl]),
            "cen": cen,
            "labf": lloc.astype(np.float32),
            "labi": lloc.astype(np.int32),
            "winl": np.ascontiguousarray(winl),
        })

    nc = _get_program()
    res = run_bass_kernel_spmd(nc, in_maps, core_ids=list(range(M)), **RUN_KWARGS)
    global LAST_RESULT
    LAST_RESULT = res
    parts = np.stack([np.asarray(res.results[r]["partial"]).reshape(2)
                      for r in range(M)])
    loss = float(parts[:, 0].sum())
    nvs = float(parts[:, 1].sum())
    out = loss / nvs if nvs > 0 else loss
    return np.array(out, dtype=np.float32)
